# revision 6
# baseline (speedup 1.0000x reference)
"""Bass/Trainium2 8-core kernel for nn_MultiHeadAttention_43155831390829.

Sharding: core c -> (batch b = c//4, head group g = c%4 i.e. heads 4g..4g+3).
Each core:
  - computes Q^T, K^T ([feat, seq] layout) and V ([seq, feat]) projections for
    its (batch, head-group) on chip,
  - runs causal attention for its 4 heads over the full 2048-seq in S^T layout
    (scores [key, query]); softmax has no max-subtraction (scores are ~N(0,1)
    for this problem's data) and the denominator comes from a ones-column
    appended to V in the P@V matmul,
  - AllToAll (two 4-core groups, one per batch) redistributes attention
    outputs so every core holds all 16 heads for a 512-wide query slice,
  - out-projection produces final^T [1024, 512] which the host transposes and
    stitches into the full [2, 2048, 1024] output.
"""

import sys

sys.path.insert(0, "/opt/trn_rl_repo")

import ml_dtypes
import numpy as np

import concourse.bass as bass
import concourse.mybir as mybir
import concourse.tile as tile
from concourse import bacc
from concourse.bass_utils import run_bass_kernel_spmd

N_CORES = 8
HIDDEN = 1024
HEADS = 16
HEAD_DIM = 64
BSZ = 2
SEQ = 2048
SCALE = HEAD_DIM ** (-0.5)
LOCAL_HEADS = 4  # heads per core
LOCAL_INNER = LOCAL_HEADS * HEAD_DIM  # 256
QSLICE = SEQ // 4  # 512, query columns per core after AllToAll

DT = mybir.dt.bfloat16
F32 = mybir.dt.float32
BF16 = ml_dtypes.bfloat16

_CACHED_NC = None


def build_nc():
    nc = bacc.Bacc("TRN2", target_bir_lowering=False, debug=False, num_devices=N_CORES)

    xqT = nc.dram_tensor("xqT", [HIDDEN, SEQ], DT, kind="ExternalInput")
    xkT = nc.dram_tensor("xkT", [HIDDEN, SEQ], DT, kind="ExternalInput")
    xvT = nc.dram_tensor("xvT", [HIDDEN, SEQ], DT, kind="ExternalInput")
    wq = nc.dram_tensor("wq", [HIDDEN, LOCAL_INNER], DT, kind="ExternalInput")
    wk = nc.dram_tensor("wk", [HIDDEN, LOCAL_INNER], DT, kind="ExternalInput")
    wv = nc.dram_tensor("wv", [HIDDEN, LOCAL_INNER], DT, kind="ExternalInput")
    wo = nc.dram_tensor("wo", [HIDDEN, HIDDEN], DT, kind="ExternalInput")
    masks = nc.dram_tensor("masks", [128, 512], F32, kind="ExternalInput")
    outT = nc.dram_tensor("outT", [HIDDEN, QSLICE], F32, kind="ExternalOutput")

    # collective bounce buffers (internal DRAM); 8-core AllToAll: block d of
    # cc_in (rows 256d..256d+256) goes to core d; cc_out row-block s holds the
    # 256 inner dims of source core s for THIS core's 256-wide q slice.
    cc_in = nc.dram_tensor("cc_in", [2048, 256], DT)
    cc_out = nc.dram_tensor("cc_out", [2048, 256], DT)

    with tile.TileContext(nc) as tc:
        with (
            tc.tile_pool(name="const", bufs=1) as cp,
            tc.tile_pool(name="work", bufs=3) as wp,
            tc.tile_pool(name="eps", bufs=2) as ep,
            tc.tile_pool(name="ps_proj", bufs=2, space="PSUM") as pj,
            tc.tile_pool(name="ps_st", bufs=2, space="PSUM") as pst,
            tc.tile_pool(name="ps_pv", bufs=2, space="PSUM") as ppv,
        ):
            # ---- persistent SBUF tiles -------------------------------------
            wq_sb = [cp.tile([128, LOCAL_INNER], DT, tag=f"wq{k}", name=f"wq_sb{k}") for k in range(8)]
            wk_sb = [cp.tile([128, LOCAL_INNER], DT, tag=f"wk{k}", name=f"wk_sb{k}") for k in range(8)]
            wv_sb = [cp.tile([128, LOCAL_INNER], DT, tag=f"wv{k}", name=f"wv_sb{k}") for k in range(8)]
            wo_sb = [cp.tile([128, HIDDEN], DT, tag=f"wo{k}", name=f"wo_sb{k}") for k in range(8)]
            xq_sb = [cp.tile([128, SEQ], DT, tag=f"xq{k}", name=f"xq_sb{k}") for k in range(8)]
            xk_sb = [cp.tile([128, SEQ], DT, tag=f"xk{k}", name=f"xk_sb{k}") for k in range(8)]
            xv_sb = [cp.tile([128, SEQ], DT, tag=f"xv{k}", name=f"xv_sb{k}") for k in range(8)]
            mask_sb = cp.tile([128, 512], F32, tag="mask")
            kT_sb = [cp.tile([128, SEQ], DT, tag=f"kT{i}", name=f"kT_sb{i}") for i in range(2)]
            qT_sb = [cp.tile([128, SEQ], DT, tag=f"qT{i}", name=f"qT_sb{i}") for i in range(2)]
            v_sb = [cp.tile([128, LOCAL_HEADS * 65], DT, tag=f"v{t}", name=f"v_sb{t}") for t in range(16)]
            attnT_sb = [cp.tile([128, SEQ], DT, tag=f"at{i}", name=f"attnT_sb{i}") for i in range(2)]
            agx_sb = [cp.tile([128, 256], DT, tag=f"ag{k}", name=f"agx_sb{k}") for k in range(16)]

            # ---- input DMAs ------------------------------------------------
            for k in range(8):
                nc.sync.dma_start(wk_sb[k][:, :], wk[128 * k : 128 * k + 128, :])
                nc.sync.dma_start(xk_sb[k][:, :], xkT[128 * k : 128 * k + 128, :])
            for k in range(8):
                nc.sync.dma_start(wv_sb[k][:, :], wv[128 * k : 128 * k + 128, :])
                nc.sync.dma_start(xv_sb[k][:, :], xvT[128 * k : 128 * k + 128, :])
            for k in range(8):
                nc.sync.dma_start(wq_sb[k][:, :], wq[128 * k : 128 * k + 128, :])
                nc.sync.dma_start(xq_sb[k][:, :], xqT[128 * k : 128 * k + 128, :])
            nc.sync.dma_start(mask_sb[:, :], masks[:, :])
            for k in range(8):
                nc.sync.dma_start(wo_sb[k][:, :], wo[128 * k : 128 * k + 128, :])

            # ---- K^T projection: kT = wk^T @ xkT  [256, 2048] --------------
            for m in range(2):
                for n in range(4):
                    ps = pj.tile([128, 512], F32, tag="proj")
                    for k in range(8):
                        nc.tensor.matmul(
                            ps[:, :],
                            lhsT=wk_sb[k][:, 128 * m : 128 * m + 128],
                            rhs=xk_sb[k][:, 512 * n : 512 * n + 512],
                            start=(k == 0),
                            stop=(k == 7),
                        )
                    nc.scalar.copy(kT_sb[m][:, 512 * n : 512 * n + 512], ps[:, :])

            # ---- V projection (row layout): v = xv @ wv [2048, 256] --------
            for rt in range(16):
                ps = pj.tile([128, 512], F32, tag="proj")
                for k in range(8):
                    nc.tensor.matmul(
                        ps[:, 0:LOCAL_INNER],
                        lhsT=xv_sb[k][:, 128 * rt : 128 * rt + 128],
                        rhs=wv_sb[k][:, :],
                        start=(k == 0),
                        stop=(k == 7),
                    )
                # fill with ones first; V columns overwrite, col 64 of each
                # 65-wide head block stays 1.0 (softmax denominator trick)
                nc.vector.memset(v_sb[rt][:, :], 1.0)
                nc.vector.tensor_copy(
                    v_sb[rt][:, :].rearrange("p (h x) -> p h x", x=65)[:, :, 0:64],
                    ps[:, 0:LOCAL_INNER].rearrange("p (h x) -> p h x", x=64),
                )

            # ---- Q^T projection --------------------------------------------
            for m in range(2):
                for n in range(4):
                    ps = pj.tile([128, 512], F32, tag="proj")
                    for k in range(8):
                        nc.tensor.matmul(
                            ps[:, :],
                            lhsT=wq_sb[k][:, 128 * m : 128 * m + 128],
                            rhs=xq_sb[k][:, 512 * n : 512 * n + 512],
                            start=(k == 0),
                            stop=(k == 7),
                        )
                    nc.scalar.copy(qT_sb[m][:, 512 * n : 512 * n + 512], ps[:, :])

            # ---- attention, head-sequential --------------------------------
            for hp in range(LOCAL_HEADS):
                ti, poff = hp // 2, 64 * (hp % 2)
                items = [(k, t) for k in range(8) for t in range(2 * k + 2)]
                groups = [items[i : i + 4] for i in range(0, len(items), 4)]
                pv = None
                for group in groups:
                    ps = pst.tile([128, 1024], F32, tag="st")
                    for j, (k, t) in enumerate(group):
                        nc.tensor.matmul(
                            ps[:, 256 * j : 256 * j + 256],
                            lhsT=kT_sb[ti][poff : poff + 64, 128 * t : 128 * t + 128],
                            rhs=qT_sb[ti][poff : poff + 64, 256 * k : 256 * k + 256],
                            start=True,
                            stop=True,
                        )
                        if t >= 2 * k:  # diagonal tile -> additive causal mask
                            moff = 0 if t == 2 * k else 256
                            nc.vector.tensor_tensor(
                                ps[:, 256 * j : 256 * j + 256],
                                ps[:, 256 * j : 256 * j + 256],
                                mask_sb[:, moff : moff + 256],
                                op=mybir.AluOpType.add,
                            )
                    pT = wp.tile([128, 1024], DT, tag="pT")
                    nc.scalar.activation(
                        pT[:, 0 : 256 * len(group)],
                        ps[:, 0 : 256 * len(group)],
                        mybir.ActivationFunctionType.Exp,
                        scale=SCALE,
                    )
                    for j, (k, t) in enumerate(group):
                        if t == 0:
                            pv = ppv.tile([65, 256], F32, tag="pv")
                        nc.tensor.matmul(
                            pv[:, :],
                            lhsT=v_sb[t][:, 65 * hp : 65 * hp + 65],
                            rhs=pT[:, 256 * j : 256 * j + 256],
                            start=(t == 0),
                            stop=(t == 2 * k + 1),
                        )
                        if t == 2 * k + 1:  # chunk done -> normalize
                            rcp = ep.tile([1, 256], F32, tag="rcp")
                            nc.vector.reciprocal(rcp[0:1, :], pv[64:65, :])
                            rcpb = ep.tile([64, 256], F32, tag="rcpb")
                            nc.gpsimd.partition_broadcast(
                                rcpb[:, :], rcp[0:1, :], channels=64
                            )
                            nc.vector.tensor_tensor(
                                attnT_sb[ti][poff : poff + 64, 256 * k : 256 * k + 256],
                                pv[0:64, :],
                                rcpb[:, :],
                                op=mybir.AluOpType.mult,
                            )

            # ---- AllToAll: redistribute attn outputs -----------------------
            # cc_in block d (rows 256d..256d+256) = attnT[:, 256d:256d+256]
            for d in range(8):
                for ti in range(2):
                    nc.sync.dma_start(
                        cc_in[256 * d + 128 * ti : 256 * d + 128 * ti + 128, :],
                        attnT_sb[ti][:, 256 * d : 256 * d + 256],
                    )
            nc.gpsimd.collective_compute(
                "AllToAll",
                mybir.AluOpType.bypass,
                replica_groups=[list(range(N_CORES))],
                ins=[cc_in.ap().opt()],
                outs=[cc_out.ap().opt()],
            )
            # cc_out rows 0..1023 = batch0 full inner, 1024..2047 = batch1,
            # both for this core's q columns [256c, 256c+256) of each batch.
            for k in range(16):
                nc.sync.dma_start(agx_sb[k][:, :], cc_out[128 * k : 128 * k + 128, :])

            # ---- out-projection: outT[:, 256b:256b+256] = wo^T @ attnT_b ---
            for m in range(8):
                ps = pj.tile([128, 512], F32, tag="proj")
                for bb in range(2):
                    for k in range(8):
                        nc.tensor.matmul(
                            ps[:, 256 * bb : 256 * bb + 256],
                            lhsT=wo_sb[k][:, 128 * m : 128 * m + 128],
                            rhs=agx_sb[8 * bb + k][:, :],
                            start=(k == 0),
                            stop=(k == 7),
                        )
                ob = wp.tile([128, 512], F32, tag="ob")
                nc.scalar.copy(ob[:, :], ps[:, :])
                nc.sync.dma_start(outT[128 * m : 128 * m + 128, :], ob[:, :])

    nc.compile()
    return nc


def _make_masks():
    l = np.arange(128)[:, None]
    qr = np.arange(256)[None, :]
    m0 = np.where(l <= qr, 0.0, -30000.0).astype(np.float32)
    m1 = np.where(l + 128 <= qr, 0.0, -30000.0).astype(np.float32)
    return np.concatenate([m0, m1], axis=1)  # [128, 512]


def make_in_maps(query, key, value, w_q, w_k, w_v, w_o):
    masks = _make_masks()
    in_maps = []
    for c in range(N_CORES):
        b, g = c // 4, c % 4
        cols = slice(LOCAL_INNER * g, LOCAL_INNER * (g + 1))
        in_maps.append(
            {
                "xqT": np.ascontiguousarray(query[b].T).astype(BF16),
                "xkT": np.ascontiguousarray(key[b].T).astype(BF16),
                "xvT": np.ascontiguousarray(value[b].T).astype(BF16),
                "wq": np.ascontiguousarray(w_q[:, cols]).astype(BF16),
                "wk": np.ascontiguousarray(w_k[:, cols]).astype(BF16),
                "wv": np.ascontiguousarray(w_v[:, cols]).astype(BF16),
                "wo": np.ascontiguousarray(w_o).astype(BF16),
                "masks": masks,
            }
        )
    return in_maps


def assemble_output(results):
    out = np.empty((BSZ, SEQ, HIDDEN), dtype=np.float32)
    for c in range(N_CORES):
        sl = slice(256 * c, 256 * c + 256)
        out[0, sl, :] = results[c]["outT"][:, 0:256].T
        out[1, sl, :] = results[c]["outT"][:, 256:512].T
    return out


def kernel(query, key, value, w_q, w_k, w_v, w_o):
    global _CACHED_NC
    if _CACHED_NC is None:
        _CACHED_NC = build_nc()
    in_maps = make_in_maps(query, key, value, w_q, w_k, w_v, w_o)
    res = run_bass_kernel_spmd(_CACHED_NC, in_maps, core_ids=list(range(N_CORES)))
    return assemble_output(res.results)
